# revision 21
# baseline (speedup 1.0000x reference)
"""Trainium2 Bass kernel for nn_AdaptiveAggregationLayer (GNN message passing).

Strategy (8 NeuronCores, no collectives needed):
  - Destination nodes sharded across cores (12500 per core); x replicated so
    each core gathers source features from its own HBM copy.
  - Edges bucketed host-side by (dest-core, dest-window-of-128, src-region);
    per-bucket source rows fetched with gpsimd dma_gather (int16 local idx).
  - segment_sum via TensorE: per 128-edge block, a one-hot selection matrix S
    (built on DVE with iota/is_equal) and matmul accumulation into PSUM:
    nbsum[d, f] += S.T @ gathered_feats.
  - Dense epilogue per 128-node window: mean = nbsum * invdeg;
    transposes of x_own/mean chunks via PE; h_mean and h_concat as
    PSUM-accumulated matmuls against host-prepared stacked weights
    (0.5 folded into W_mean; W_ego/W_nb block-diagonal); bias via K=1 matmul;
    gate mix on ACT/DVE; DMA out.
  - Degrees (pure graph structure) and edge binning/padding are host-side
    sharding prep; all feature math runs on device.
"""
import math
import numpy as np

import concourse.bass as bass
import concourse.bacc as bacc
import concourse.mybir as mybir
from concourse import tile
from concourse.bass_utils import run_bass_kernel_spmd

F32 = mybir.dt.float32
F32R = mybir.dt.float32r
BF16 = mybir.dt.bfloat16
I16 = mybir.dt.int16

# Problem configuration (hardcoded per spec).
CFG = dict(
    N=100000,
    F=256,
    CORES=8,
    REG=4,  # source regions (int16 gather index must stay < 32768)
)

# gather/compute dtype for the edge-feature stream ("f32" or "bf16")
GATHER_MODE = "bf16"
import os
S_DIRECT_FP8 = os.environ.get("S_DIRECT_FP8", "1") == "1"

LAST_EXEC_NS = None
LAST_RESULTS = None


def _derive(cfg):
    N, CORES = cfg["N"], cfg["CORES"]
    NPC = N // CORES
    NWIN = math.ceil(NPC / 128)
    NPCP = NWIN * 128
    REGSZ = math.ceil(N / cfg["REG"])
    assert REGSZ < 32768
    return NPC, NWIN, NPCP, REGSZ


def _host_prep(x, edge_index, delta_agg, cfg, build_s=False):
    """Bucket/pad edges, compute degrees, build per-core device arrays."""
    N, F, CORES, REG = cfg["N"], cfg["F"], cfg["CORES"], cfg["REG"]
    NPC, NWIN, NPCP, REGSZ = _derive(cfg)

    row = np.asarray(edge_index[0]).astype(np.int64)
    col = np.asarray(edge_index[1]).astype(np.int64)

    c = row // NPC
    loc = row - c * NPC
    w = loc >> 7
    d = (loc & 127).astype(np.float32)
    b = col // REGSZ
    lcol = (col - b * REGSZ).astype(np.int16)

    bucket = (c * NWIN + w) * REG + b
    order = np.argsort(bucket, kind="stable")
    lcol_s = lcol[order]
    d_s = d[order]

    counts = np.bincount(bucket, minlength=CORES * NWIN * REG).reshape(
        CORES, NWIN, REG
    )
    nblk = ((counts + 127) // 128).max(axis=0)  # [NWIN, REG] shared shape
    nblk[:, 0] = np.maximum(nblk[:, 0], 1)
    Tw = nblk.sum(axis=1)  # [NWIN]
    W0 = np.zeros(NWIN, dtype=np.int64)
    W0[1:] = np.cumsum(Tw)[:-1]
    TOTBLK = int(Tw.sum())
    blk0 = np.zeros((NWIN, REG), dtype=np.int64)
    blk0[:, 1:] = np.cumsum(nblk, axis=1)[:, :-1]
    blk0 += W0[:, None]

    ends = np.cumsum(counts.reshape(-1)).reshape(CORES, NWIN, REG)
    starts = ends - counts

    deg = np.bincount(row, minlength=N).astype(np.float32)
    invdeg = 1.0 / np.maximum(deg, 1.0)

    delta = np.asarray(delta_agg).astype(np.float32)

    per_core = []
    for ci in range(CORES):
        src_idx = np.zeros((16, TOTBLK * 8), np.int16)
        dst_rel = np.full((128, TOTBLK), -1.0, np.float32)
        for wi in range(NWIN):
            for bi in range(REG):
                nb = int(nblk[wi, bi])
                if nb == 0:
                    continue
                P = nb * 128
                s = int(starts[ci, wi, bi])
                k = int(counts[ci, wi, bi])
                o = int(blk0[wi, bi])
                ia = np.zeros(P, np.int16)
                ia[:k] = lcol_s[s : s + k]
                dr = np.full(P, -1.0, np.float32)
                dr[:k] = d_s[s : s + k]
                src_idx[:, o * 8 : (o + nb) * 8] = ia.reshape(nb * 8, 16).T
                dst_rel[:, o : o + nb] = dr.reshape(nb, 128).T

        xo = np.zeros((NPCP, F), np.float32)
        xo[:NPC] = np.asarray(x)[ci * NPC : (ci + 1) * NPC]
        ivc = np.zeros(NPCP, np.float32)
        ivc[:NPC] = invdeg[ci * NPC : (ci + 1) * NPC]
        dlc = np.zeros(NPCP, np.float32)
        dlc[:NPC] = delta[ci * NPC : (ci + 1) * NPC]
        entry = dict(
            src_idx=np.tile(src_idx, (8, 1)),  # replicated for 8 Q7 cores
            dst_rel=dst_rel,
            x_own=xo,
            invdeg=ivc.reshape(NWIN, 128).T.copy(),
            delta=dlc.reshape(NWIN, 128).T.copy(),
        )
        if build_s:
            e_idx, blk_idx = np.nonzero(dst_rel >= 0)
            dv = dst_rel[e_idx, blk_idx].astype(np.int64)
            S = np.zeros((128, TOTBLK * 128), dtype=mybir.dt.np(mybir.dt.float8e4))
            S[e_idx, blk_idx * 128 + dv] = 1
            entry["S"] = S
        per_core.append(entry)

    maxcnt = counts.max(axis=0)  # [NWIN, REG] exact idx count per bucket
    maxcnt = np.maximum(maxcnt, (nblk > 0).astype(maxcnt.dtype))
    shape = dict(nblk=nblk, Tw=Tw, W0=W0, blk0=blk0, TOTBLK=TOTBLK, maxcnt=maxcnt)
    return per_core, shape


def _build_graph(cfg, shape, gate_weight, gate_bias, gather_mode):
    N, F, REG = cfg["N"], cfg["F"], cfg["REG"]
    NPC, NWIN, NPCP, REGSZ = _derive(cfg)
    nblk, Tw, W0, blk0, TOTBLK = (
        shape["nblk"],
        shape["Tw"],
        shape["W0"],
        shape["blk0"],
        shape["TOTBLK"],
    )
    maxcnt = shape["maxcnt"]
    bf = gather_mode == "bf16"
    gdt = BF16 if bf else F32   # gathered features / S / iota
    cdt = BF16 if bf else F32   # phase-B matmul operand dtype

    nc = bacc.Bacc("TRN2", target_bir_lowering=False, debug=False,
                   num_swdge_queues=4)

    x_d = nc.dram_tensor("x", [N, F], gdt, kind="ExternalInput")
    xown_d = nc.dram_tensor("x_own", [NPCP, F], cdt, kind="ExternalInput")
    srcidx_d = nc.dram_tensor("src_idx", [128, TOTBLK * 8], I16, kind="ExternalInput")
    dstrel_d = None if bf else nc.dram_tensor(
        "dst_rel", [128, TOTBLK], F32, kind="ExternalInput")
    invd_d = nc.dram_tensor("invdeg", [128, NWIN], F32, kind="ExternalInput")
    delt_d = nc.dram_tensor("delta", [128, NWIN], F32, kind="ExternalInput")
    wc_d = nc.dram_tensor("WC", [512, 2 * F], cdt, kind="ExternalInput")
    bc2_d = nc.dram_tensor("bC", [1, 2 * F], cdt, kind="ExternalInput")
    if bf:
        s_d = nc.dram_tensor("S", [128, TOTBLK * 128], mybir.dt.float8e4, kind="ExternalInput")
    else:
        iota_d = nc.dram_tensor("iota", [128, 128], gdt, kind="ExternalInput")
    idn_d = nc.dram_tensor("ident", [128, 128], cdt, kind="ExternalInput")
    ones_d = nc.dram_tensor("ones", [1, 128], cdt, kind="ExternalInput")
    out_d = nc.dram_tensor("out", [NPCP, F], F32, kind="ExternalOutput")

    AT = mybir.ActivationFunctionType
    OP = mybir.AluOpType

    with tile.TileContext(nc) as tc:
        with (
            tc.tile_pool(name="const", bufs=1) as cpool,
            tc.tile_pool(name="main", bufs=2) as pool,
            tc.tile_pool(name="stream", bufs=3) as spool3,
            tc.tile_pool(name="spool", bufs=4) as spool,
            tc.tile_pool(name="psum", bufs=2, space="PSUM") as ppool,
            tc.tile_pool(name="psum3", bufs=3, space="PSUM") as ppool3,
        ):
            wc = cpool.tile([128, 4, 2 * F], cdt, tag="wc")
            for k in range(4):
                nc.sync.dma_start(out=wc[:, k, :], in_=wc_d[k * 128 : (k + 1) * 128, :])
            bc2 = cpool.tile([1, 2 * F], cdt, tag="bc2")
            nc.sync.dma_start(out=bc2[:, :], in_=bc2_d[:, :])
            ones = cpool.tile([1, 128], cdt, tag="ones")
            nc.sync.dma_start(out=ones[:, :], in_=ones_d[:, :])
            if not bf:
                iota = cpool.tile([128, 128], gdt, tag="iota")
                nc.sync.dma_start(out=iota[:, :], in_=iota_d[:, :])
            idn = cpool.tile([128, 128], cdt, tag="idn")
            nc.sync.dma_start(out=idn[:, :], in_=idn_d[:, :])
            if not bf:
                dstrel = cpool.tile([128, TOTBLK], F32, tag="dstrel")
                nc.sync.dma_start(out=dstrel[:, :], in_=dstrel_d[:, :])
            invd = cpool.tile([128, NWIN], F32, tag="invd")
            nc.sync.dma_start(out=invd[:, :], in_=invd_d[:, :])
            delt = cpool.tile([128, NWIN], F32, tag="delt")
            nc.sync.dma_start(out=delt[:, :], in_=delt_d[:, :])

            g = cpool.tile([128, NWIN], F32, tag="g")
            nc.scalar.activation(
                g[:, :], delt[:, :], AT.Sigmoid,
                bias=float(gate_bias), scale=float(gate_weight),
            )
            omg = cpool.tile([128, NWIN], F32, tag="omg")
            nc.vector.tensor_scalar(omg[:, :], g[:, :], -1.0, 1.0, OP.mult, OP.add)

            gq = [0]
            for w in range(NWIN):
                T = int(Tw[w])
                w0 = int(W0[w])
                idxw = spool3.tile([128, T * 8], I16, tag="idxw")
                nc.sync.dma_start(
                    out=idxw[:, :], in_=srcidx_d[:, w0 * 8 : (w0 + T) * 8]
                )
                gath = spool3.tile([128, T, F], gdt, tag="gath")
                for bi in range(REG):
                    nb = int(nblk[w, bi])
                    if nb == 0:
                        continue
                    o = int(blk0[w, bi]) - w0
                    ni = int(maxcnt[w, bi])
                    if ni % 128 != 0:
                        nc.vector.memset(gath[:, o + nb - 1, :], 0.0)
                    nc.gpsimd.dma_gather(
                        gath[:, o : o + nb, :],
                        x_d[bi * REGSZ : min((bi + 1) * REGSZ, N), :],
                        idxw[:, o * 8 : o * 8 + (ni + 15) // 16],
                        ni,
                        ni,
                        F,
                        single_packet=False,
                        queue_num=gq[0] % 4,
                    )
                    gq[0] += 1
                nbs = ppool3.tile([128, F], F32, tag="nbsum")
                if bf:
                    swin8 = spool3.tile([128, T * 128], mybir.dt.float8e4, tag="swin8")
                    nc.sync.dma_start(
                        out=swin8[:, :], in_=s_d[:, w0 * 128 : (w0 + T) * 128]
                    )
                    if S_DIRECT_FP8:
                        swin = swin8
                    else:
                        swin = spool3.tile([128, T * 128], BF16, tag="swin")
                        nc.vector.tensor_copy(swin[:, :], swin8[:, :])
                    for t in range(T):
                        nc.tensor.matmul(
                            nbs[:, :],
                            swin[:, t * 128 : (t + 1) * 128],
                            gath[:, t, :],
                            start=(t == 0),
                            stop=(t == T - 1),
                        )
                else:
                    for t in range(T):
                        S = spool.tile([128, 128], gdt, tag="S")
                        nc.vector.tensor_scalar(
                            S[:, :], iota[:, :], dstrel[:, w0 + t : w0 + t + 1],
                            None, OP.is_equal,
                        )
                        nc.tensor.matmul(
                            nbs[:, :],
                            S[:, :],
                            gath[:, t, :],
                            start=(t == 0),
                            stop=(t == T - 1),
                        )
                mean = pool.tile([128, F], cdt, tag="mean")
                nc.scalar.activation(
                    mean[:, :], nbs[:, :], AT.Copy, scale=invd[:, w : w + 1]
                )
                xo = spool3.tile([128, F], cdt, tag="xo")
                nc.sync.dma_start(
                    out=xo[:, :], in_=xown_d[w * 128 : (w + 1) * 128, :]
                )
                tp = ppool.tile([128, 512], cdt, tag="tps")
                nc.tensor.transpose(tp[:, 0:128], xo[:, 0:128], idn[:, :])
                nc.tensor.transpose(tp[:, 128:256], xo[:, 128:256], idn[:, :])
                nc.tensor.transpose(tp[:, 256:384], mean[:, 0:128], idn[:, :])
                nc.tensor.transpose(tp[:, 384:512], mean[:, 128:256], idn[:, :])
                lhs = pool.tile([128, 512], cdt, tag="lhs")
                nc.vector.tensor_copy(lhs[:, 0:256], tp[:, 0:256])
                nc.vector.tensor_copy(lhs[:, 256:512], tp[:, 256:512])

                hcomb = ppool.tile([128, 2 * F], F32, tag="hcomb")
                nc.tensor.matmul(
                    hcomb[:, :], ones[:, :], bc2[:, :],
                    start=True, stop=False,
                )
                for k in range(4):
                    nc.tensor.matmul(
                        hcomb[:, :],
                        lhs[:, k * 128 : (k + 1) * 128],
                        wc[:, k, :],
                        start=False,
                        stop=(k == 3),
                    )
                av = pool.tile([128, F], F32, tag="av")
                nc.scalar.activation(
                    av[:, :], hcomb[:, 0:F], AT.Copy, scale=omg[:, w : w + 1]
                )
                bv = pool.tile([128, F], F32, tag="bv")
                nc.vector.tensor_scalar(bv[:, :], hcomb[:, F : 2 * F], g[:, w : w + 1], None, OP.mult)
                ot = pool.tile([128, F], F32, tag="ot")
                nc.vector.tensor_tensor(ot[:, :], av[:, :], bv[:, :], op=OP.add)
                nc.sync.dma_start(
                    out=out_d[w * 128 : (w + 1) * 128, :], in_=ot[:, :]
                )
    nc.compile()
    return nc


def _make_weight_arrays(W_mean, b_mean, W_ego, b_ego, W_nb, b_nb, cfg, gather_mode):
    F = cfg["F"]
    EGO = W_ego.shape[1]
    W_mean = np.asarray(W_mean, np.float32)
    WA = np.concatenate([0.5 * W_mean, 0.5 * W_mean], axis=0)
    WB = np.zeros((2 * F, F), np.float32)
    WB[0:F, 0:EGO] = np.asarray(W_ego, np.float32)
    WB[F : 2 * F, EGO:F] = np.asarray(W_nb, np.float32)
    bm = np.asarray(b_mean, np.float32)[None, :]
    bcat = np.concatenate(
        [np.asarray(b_ego, np.float32), np.asarray(b_nb, np.float32)]
    )[None, :]
    WC = np.concatenate([WA, WB], axis=1)          # [512, 512]
    bC = np.concatenate([bm, bcat], axis=1)        # [1, 512]
    npdt = np.float32 if gather_mode == "f32" else mybir.dt.np(BF16)
    iota = np.broadcast_to(np.arange(128, dtype=np.float32), (128, 128)).astype(npdt)
    idn = np.eye(128).astype(npdt)
    ones = np.ones((1, 128)).astype(npdt)
    return (WC.astype(npdt), bC.astype(npdt), iota, idn, ones)


def run(inputs, cfg=None, gather_mode=None, trace=True, sim=False):
    """Core entry: returns (full_output, exec_time_ns)."""
    global LAST_EXEC_NS, LAST_RESULTS
    cfg = dict(CFG if cfg is None else cfg)
    gather_mode = GATHER_MODE if gather_mode is None else gather_mode
    N, F, CORES = cfg["N"], cfg["F"], cfg["CORES"]
    NPC, NWIN, NPCP, REGSZ = _derive(cfg)

    per_core, shape = _host_prep(
        inputs["x"], inputs["edge_index"], inputs["delta_agg"], cfg,
        build_s=(gather_mode == "bf16"),
    )
    WC, bC, iota, idn, ones = _make_weight_arrays(
        inputs["W_mean"], inputs["b_mean"], inputs["W_ego"], inputs["b_ego"],
        inputs["W_nb"], inputs["b_nb"], cfg, gather_mode,
    )
    gnp = np.float32 if gather_mode == "f32" else mybir.dt.np(BF16)
    xg = np.ascontiguousarray(np.asarray(inputs["x"]).astype(gnp))

    nc = _build_graph(
        cfg, shape, float(inputs["gate_weight"]), float(inputs["gate_bias"]),
        gather_mode,
    )

    in_maps = []
    for ci in range(CORES):
        pc = per_core[ci]
        m = {
            "x": xg,
            "x_own": pc["x_own"].astype(gnp),
            "src_idx": pc["src_idx"],
            "invdeg": pc["invdeg"],
            "delta": pc["delta"],
            "WC": WC,
            "bC": bC,
            "ident": idn,
            "ones": ones,
        }
        if gather_mode == "bf16":
            m["S"] = pc["S"]
        else:
            m["dst_rel"] = pc["dst_rel"]
            m["iota"] = iota
        in_maps.append(m)

    if sim:
        from concourse import bass_interp

        mcs = bass_interp.MultiCoreSim(nc, CORES)
        for ci in range(CORES):
            for k, v in in_maps[ci].items():
                mcs.cores[ci].tensor(k)[:] = v
        mcs.simulate(check_with_hw=False)
        outs = [
            np.array(mcs.cores[ci].mem_tensor("out")).reshape(NPCP, F)[:NPC]
            for ci in range(CORES)
        ]
        LAST_EXEC_NS = None
        return np.concatenate(outs, axis=0), None

    try:
        from bench_util import install_ntff_hook

        install_ntff_hook()
    except Exception:
        trace = False

    res = run_bass_kernel_spmd(
        nc, in_maps, core_ids=list(range(CORES)), trace=trace
    )
    LAST_RESULTS = res
    LAST_EXEC_NS = res.exec_time_ns
    outs = [res.results[ci]["out"].reshape(NPCP, F)[:NPC] for ci in range(CORES)]
    return np.concatenate(outs, axis=0), res.exec_time_ns


def kernel(**inputs) -> np.ndarray:
    out, _ = run(inputs)
    return out.astype(np.float32)
